# revision 17
# baseline (speedup 1.0000x reference)
"""Trainium2 Bass kernel for nn_CategoricalProjection (C51 categorical
projection / histogram binning), v7.

Math: out[j] = d2/dj2 R(j), R(y) = sum_a p_a relu(y - pos_a),
pos_a = clip(alpha + beta*a, 0, 50), alpha = 2.5 r + 25 - 24.75 nd,
beta = 0.99 nd.  R(y) = (y+1) P(c_y) - Vhat(c_y) with P = prefix(p),
Vhat = prefix(p*(pos+1)), c_y = #{a: pos_a < y} -- both prefixes taken as
GLOBAL running sums across group batches (constant offsets per group are
annihilated by the second difference; (y+1)P offsets are linear in y).

Per 128-row x 8-group macro:
  PE matmul (lhsT = [rT; ndT], const AFF) -> posext[p, (g,c)] (affine tile)
  Act: pt16 = 320*p (fp16), pos16 = posext - 50.5 (fp16)
  DVE custom scans: PPS (uint16, 4-group batches), PVS (int16, 2-group)
  DVE custom: vcols = (1 + 54g) - clip(pos16, -50.99, -0.01)  [rint -> int16]
  GPSIMD local_scatter (last-wins): dstp[vcols] = PPS, dstv[vcols] = PVS;
    col0 of vcols = const 53 + 54g (base slot, per-group re-seed)
  DVE fills (reversed windows): JP = runmax(dstp) * jconst(k = y+1),
    R = JP - runmax(dstv)/320
  DVE stencils: out = d2 R

Sharding: pure data-parallel over batch across 8 NeuronCores.
"""
import numpy as np

import concourse.bacc as bacc
import concourse.tile as tile
from concourse import mybir
from concourse.bass import MemorySpace
from concourse.bass_utils import run_bass_kernel_spmd

# ---- problem constants ----
BS = 524288
A = 51
N_CORES = 8
ROWS = BS // N_CORES          # 65536 rows per core
P = 128
G = 8                         # groups (rows) per partition per macro
MACRO_ROWS = P * G            # 1024
M = ROWS // MACRO_ROWS        # 64 macros

CS_V = 320.0                  # Vhat fixed-point scale (uint16, 4-group)
RP = 50.0                     # extra P scale: CS_P = CS_V * RP = 16000
CS_P = CS_V * RP              # uint16, 4-group batches
NE = 54                       # dst slots per group window
W = G * 52                    # 416: pps/pvs/vcols width
WD = G * NE                   # 432: dst width

f32 = mybir.dt.float32
f16 = mybir.dt.float16
i16 = mybir.dt.int16
u16 = mybir.dt.uint16

_OPS = {}


def _register_ops():
    if _OPS:
        return _OPS
    from concourse import dve_ops as dvo
    from concourse.dve_spec import (
        Spec, Src0, Src1, C0, C1, C2, scan, AluOp, lower, minn, maxx, One,
        PageIdx,
    )
    from concourse.dve_ops import has_src1
    from concourse.dve_table_gen import DveOpSpec

    def reg(name, spec, subdim=False):
        for existing in dvo.OPS:
            if existing.name == name:
                _OPS[name] = existing
                return
        row = dvo._CUSTOM_DVE_ROW_BASE + len(dvo.OPS)
        assert row < 0x20, "custom DVE row overflow"
        dvo._SUB_OPCODE_FOR_NAME[name] = row
        shas = {}
        for ver in ("v3", "v4"):
            s = DveOpSpec(name=name, opcode=row, uops=lower(spec, ver=ver),
                          rd1_en=has_src1(spec))
            shas[ver] = s.sha(ver)
        op = dvo.DveOp(name, spec, subdim, uops_sha=shas)
        dvo.OPS.append(op)
        dvo.CUSTOM_DVE_SPECS[name] = spec
        _OPS[name] = op

    # PPS: running sum of p*320 * 50 -> uint16
    def _pps_ref(in0, in1, s0, s1, imm2):
        r = np.cumsum(in0.astype(np.float32) * np.float32(imm2), axis=-1,
                      dtype=np.float32)
        return np.clip(np.rint(r), 0, 65535)

    reg("V7_PPS", Spec(body=scan(AluOp.ADD, Src0 * C2), reference=_pps_ref))

    # PVS: running sum of p*CS_V * (clip(pos16, c0, c1) + c2) -> uint16
    # pos16 = posr - 50.5; c0 = -50.5, c1 = -0.5, c2 = 51.5
    def _pvs_ref(in0, in1, s0, s1, imm2):
        t = np.clip(in1.astype(np.float32), s0, s1) + np.float32(imm2)
        r = np.cumsum(in0.astype(np.float32) * t, axis=-1, dtype=np.float32)
        return np.clip(np.rint(r), 0, 65535)

    reg("V7_PVS",
        Spec(body=scan(AluOp.ADD, Src0 * (minn(maxx(Src1, C0), C1) + C2)),
             reference=_pvs_ref))

    # VCO: (1 + 54*page) - clip(pos16, c0, c1) -> int16 (rint)
    def _vco_ref(in0, in1, s0, s1, imm2):
        S = in0.shape[1]
        pg = 1.0 + np.float32(imm2) * np.arange(S, dtype=np.float32)
        r = pg[None, :, None] - np.clip(in0.astype(np.float32), s0, s1)
        return np.clip(np.rint(r), -32768, 32767)

    reg("V7_VCO",
        Spec(body=PageIdx(One, C2) - minn(maxx(Src0, C0), C1),
             reference=_vco_ref),
        subdim=True)

    # JPF: runmax(dstp) * jconst
    def _jpf_ref(in0, in1, s0, s1, imm2):
        return np.maximum.accumulate(in0.astype(np.float32), axis=-1) \
            * in1.astype(np.float32)

    reg("V7_JPF", Spec(body=scan(AluOp.MAX, Src0) * Src1, reference=_jpf_ref))

    # RVF: jp - runmax(dstv) * c0
    def _rvf_ref(in0, in1, s0, s1, imm2):
        return in1.astype(np.float32) \
            - np.maximum.accumulate(in0.astype(np.float32), axis=-1) * s0

    reg("V7_RVF", Spec(body=Src1 - scan(AluOp.MAX, Src0) * C0,
                       reference=_rvf_ref))
    return _OPS


def _build(n_macros=M):
    ops = _register_ops()
    nc = bacc.Bacc()
    nrows = n_macros * MACRO_ROWS
    AluOp = mybir.AluOpType
    ACTF = mybir.ActivationFunctionType

    reward_in = nc.dram_tensor("reward", [nrows, 1], f32, kind="ExternalInput")
    probs_in = nc.dram_tensor("probs", [nrows, A], f32, kind="ExternalInput")
    nd_in = nc.dram_tensor("not_done", [nrows, 1], f32, kind="ExternalInput")
    jrow_in = nc.dram_tensor("jrow", [P, 4 * NE], f32, kind="ExternalInput")
    sct_in = nc.dram_tensor("sct_t", [16, nrows // G], f32,
                            kind="ExternalInput")
    aff_in = nc.dram_tensor("aff", [16, W], f32, kind="ExternalInput")
    vc0_in = nc.dram_tensor("vc0", [P, G], i16, kind="ExternalInput")
    out_t = nc.dram_tensor("out", [nrows, A], f32, kind="ExternalOutput")

    # row(m, p, g) = m*1024 + p*8 + g
    pr = probs_in[:].rearrange("(m p g) c -> m p (g c)", m=n_macros, p=P, g=G)
    outr = out_t[:].rearrange("(m p g) c -> m p (g c)", m=n_macros, p=P, g=G)


    with tile.TileContext(nc) as tc:
        with tc.tile_pool(name="consts", bufs=1) as cpool, \
             tc.tile_pool(name="work", bufs=3) as pool, \
             tc.tile_pool(name="dsts", bufs=3) as dpool, \
             tc.tile_pool(name="psum", bufs=3, space=MemorySpace.PSUM) as ppool:
            jrow = cpool.tile([P, 4 * NE], f32)
            nc.sync.dma_start(out=jrow[:], in_=jrow_in[:])
            aff = cpool.tile([16, W], f32)
            nc.sync.dma_start(out=aff[:], in_=aff_in[:])
            # all per-row scalars for all macros, transposed: [16, M*128]
            sct = cpool.tile([16, n_macros * P], f32)
            nc.sync.dma_start(out=sct[:], in_=sct_in[:])

            # warm up rotating buffers that carry persistent constants
            for b in range(3):
                pt = pool.tile([P, W], f32, name=f"pt32_w{b}", tag="pt32")
                nc.vector.memset(pt[:], 0)
            for b in range(3):
                vc = pool.tile([P, W], i16, name=f"vcols_w{b}", tag="vcols")
                vcr = vc[:].rearrange("p (g x) -> p g x", g=G)
                nc.sync.dma_start(
                    out=vcr[:, :, 0:1],
                    in_=vc0_in[:].rearrange("p (g o) -> p g o", o=1))

            jrow_r = jrow[:].rearrange("p (g x) -> p g x", g=4)
            prev = None
            for mi in range(n_macros + 1):
                if mi < n_macros:
                    ptile = pool.tile([P, G * A], f32, tag="ptile")
                    nc.sync.dma_start(out=ptile[:], in_=pr[mi])

                    pos_ps = ppool.tile([P, W], f32, tag="pos")
                    nc.tensor.matmul(pos_ps[:], sct[:, mi * P:(mi + 1) * P],
                                     aff[:], start=True, stop=True)

                    pt32 = pool.tile([P, W], f32, tag="pt32")
                    pt32_r = pt32[:].rearrange("p (g x) -> p g x", g=G)
                    nc.scalar.activation(pt32_r[:, :, 1:52], ptile[:],
                                         ACTF.Copy, bias=0.0, scale=CS_V)
                    pos32 = pool.tile([P, W], f32, tag="pos32")
                    nc.scalar.activation(pos32[:], pos_ps[:], ACTF.Copy,
                                         bias=-25.5, scale=1.0)

                    pps = pool.tile([P, W], u16, tag="pps")
                    for b in range(2):
                        h = slice(b * 208, (b + 1) * 208)
                        nc.vector._custom_dve(ops["V7_PPS"], out=pps[:, h],
                                              in0=pt32[:, h], imm2=RP)
                    pvs = pool.tile([P, W], u16, tag="pvs")
                    for b in range(2):
                        h = slice(b * 208, (b + 1) * 208)
                        nc.vector._custom_dve(ops["V7_PVS"], out=pvs[:, h],
                                              in0=pt32[:, h], in1=pos32[:, h],
                                              s0=-50.5, s1=-0.5, imm2=51.5)
                    vcols = pool.tile([P, W], i16, tag="vcols")
                    vcr = vcols[:].rearrange("p (g x) -> p g x", g=G)
                    p32r = pos32[:].rearrange("p (g x) -> p g x", g=G)
                    nc.vector._custom_dve(ops["V7_VCO"], out=vcr[:, :, 1:52],
                                          in0=p32r[:, :, 1:52],
                                          s0=-50.99, s1=-0.01, imm2=float(NE))

                    dstp = dpool.tile([P, WD], u16, tag="dstp")
                    nc.gpsimd.local_scatter(dstp[:], pps[:], vcols[:],
                                            channels=P, num_elems=WD,
                                            num_idxs=W)
                    dstv = dpool.tile([P, WD], u16, tag="dstv")
                    nc.gpsimd.local_scatter(dstv[:], pvs[:], vcols[:],
                                            channels=P, num_elems=WD,
                                            num_idxs=W)
                    cur = (dstp, dstv, mi)

                if mi > 0:
                    dstp0, dstv0, mj = prev
                    dstp_r = dstp0[:].rearrange("p (g x) -> p g x", g=G)
                    dstv_r = dstv0[:].rearrange("p (g x) -> p g x", g=G)
                    jpf = pool.tile([P, WD], f32, tag="jpf")
                    jpf_r = jpf[:].rearrange("p (g x) -> p g x", g=G)
                    for b in range(2):
                        gs = slice(b * 4, (b + 1) * 4)
                        nc.vector._custom_dve(ops["V7_JPF"],
                                              out=jpf_r[:, gs, :],
                                              in0=dstp_r[:, gs, ::-1],
                                              in1=jrow_r[:])
                    rtile = pool.tile([P, WD], f32, tag="rtile")
                    rt_r = rtile[:].rearrange("p (g x) -> p g x", g=G)
                    for b in range(2):
                        gs = slice(b * 4, (b + 1) * 4)
                        nc.vector._custom_dve(ops["V7_RVF"],
                                              out=rt_r[:, gs, :],
                                              in0=dstv_r[:, gs, ::-1],
                                              in1=jpf_r[:, gs, :],
                                              s0=1.0 / CS_V)

                    d1t = pool.tile([P, G * 53], f32, tag="d1t")
                    d1r = d1t[:].rearrange("p (g x) -> p g x", g=G)
                    nc.vector.tensor_tensor(d1r[:], rt_r[:, :, 1:54],
                                            rt_r[:, :, 0:53], AluOp.subtract)
                    otile = pool.tile([P, G * A], f32, tag="otile")
                    otr = otile[:].rearrange("p (g x) -> p g x", g=G)
                    nc.vector.tensor_tensor(otr[:], d1r[:, :, 1:52],
                                            d1r[:, :, 0:51], AluOp.subtract)
                    nc.scalar.dma_start(out=outr[mj], in_=otile[:])

                if mi < n_macros:
                    prev = cur
    nc.compile()
    return nc


_CONSTS = None


def _const_inputs():
    global _CONSTS
    if _CONSTS is None:
        jrow = np.tile((np.arange(NE, dtype=np.float32) / np.float32(CS_P)),
                       4)
        jrow = np.tile(jrow[None, :], (P, 1)).astype(np.float32)
        # aff[q, (g, c)]: q = 0..7 -> reward row of group q: coef 2.5
        #                 q = 8..15 -> nd row: coef 0.99*(c-1) - 24.75
        # (+25 global offset folded into the pos16 activation bias)
        aff = np.zeros((16, W), dtype=np.float32)
        cc = np.arange(52, dtype=np.float32)
        for g in range(G):
            aff[g, g * 52:(g + 1) * 52] = 2.5
            aff[8 + g, g * 52:(g + 1) * 52] = 0.99 * (cc - 1.0) - 24.75
        vc0 = np.tile((53 + NE * np.arange(G, dtype=np.int16))[None, :],
                      (P, 1)).astype(np.int16)
        _CONSTS = {"jrow": jrow, "aff": aff, "vc0": vc0}
    return _CONSTS


_PROGRAM = None


def make_in_maps(reward, probs, not_done):
    consts = _const_inputs()
    in_maps = []
    for c in range(N_CORES):
        sl = slice(c * ROWS, (c + 1) * ROWS)
        rt = reward[sl].reshape(M, P, G).transpose(2, 0, 1).reshape(G, -1)
        nt = not_done[sl].reshape(M, P, G).transpose(2, 0, 1).reshape(G, -1)
        sct_t = np.ascontiguousarray(np.concatenate([rt, nt], axis=0),
                                     dtype=np.float32)
        in_maps.append({
            "reward": np.ascontiguousarray(reward[sl]),
            "probs": np.ascontiguousarray(probs[sl]),
            "not_done": np.ascontiguousarray(not_done[sl]),
            "sct_t": sct_t,
            **consts,
        })
    return in_maps


def kernel(reward, probs, not_done):
    global _PROGRAM
    reward = np.ascontiguousarray(np.asarray(reward, dtype=np.float32))
    probs = np.ascontiguousarray(np.asarray(probs, dtype=np.float32))
    not_done = np.ascontiguousarray(np.asarray(not_done, dtype=np.float32))
    assert reward.shape == (BS, 1) and probs.shape == (BS, A)

    if _PROGRAM is None:
        _PROGRAM = _build(M)
    consts = _const_inputs()

    in_maps = make_in_maps(reward, probs, not_done)
    res = run_bass_kernel_spmd(_PROGRAM, in_maps, list(range(N_CORES)))
    out = np.empty((BS, A), dtype=np.float32)
    for c in range(N_CORES):
        out[c * ROWS:(c + 1) * ROWS] = res.results[c]["out"]
    return out


# revision 18
# speedup vs baseline: 1.1534x; 1.1534x over previous
"""Trainium2 Bass kernel for nn_CategoricalProjection (C51 categorical
projection / histogram binning), v7.

Math: out[j] = d2/dj2 R(j), R(y) = sum_a p_a relu(y - pos_a),
pos_a = clip(alpha + beta*a, 0, 50), alpha = 2.5 r + 25 - 24.75 nd,
beta = 0.99 nd.  R(y) = (y+1) P(c_y) - Vhat(c_y) with P = prefix(p),
Vhat = prefix(p*(pos+1)), c_y = #{a: pos_a < y} -- both prefixes taken as
GLOBAL running sums across group batches (constant offsets per group are
annihilated by the second difference; (y+1)P offsets are linear in y).

Per 128-row x 8-group macro:
  PE matmul (lhsT = [rT; ndT], const AFF) -> posext[p, (g,c)] (affine tile)
  Act: pt16 = 320*p (fp16), pos16 = posext - 50.5 (fp16)
  DVE custom scans: PPS (uint16, 4-group batches), PVS (int16, 2-group)
  DVE custom: vcols = (1 + 54g) - clip(pos16, -50.99, -0.01)  [rint -> int16]
  GPSIMD local_scatter (last-wins): dstp[vcols] = PPS, dstv[vcols] = PVS;
    col0 of vcols = const 53 + 54g (base slot, per-group re-seed)
  DVE fills (reversed windows): JP = runmax(dstp) * jconst(k = y+1),
    R = JP - runmax(dstv)/320
  DVE stencils: out = d2 R

Sharding: pure data-parallel over batch across 8 NeuronCores.
"""
import numpy as np

import concourse.bacc as bacc
import concourse.tile as tile
from concourse import mybir
from concourse.bass import MemorySpace
from concourse.bass_utils import run_bass_kernel_spmd

# ---- problem constants ----
BS = 524288
A = 51
N_CORES = 8
ROWS = BS // N_CORES          # 65536 rows per core
P = 128
G = 8                         # groups (rows) per partition per macro
MACRO_ROWS = P * G            # 1024
M = ROWS // MACRO_ROWS        # 64 macros

CS_V = 640.0                  # Vhat fixed-point scale (uint16, 2-group)
RP = 25.0                     # extra P scale: CS_P = CS_V * RP = 16000
CS_P = CS_V * RP              # uint16, 4-group batches
NE = 54                       # dst slots per group window
W = G * 52                    # 416: pps/pvs/vcols width
WD = G * NE                   # 432: dst width

f32 = mybir.dt.float32
f16 = mybir.dt.float16
i16 = mybir.dt.int16
u16 = mybir.dt.uint16

_OPS = {}


def _register_ops():
    if _OPS:
        return _OPS
    from concourse import dve_ops as dvo
    from concourse.dve_spec import (
        Spec, Src0, Src1, C0, C1, C2, scan, AluOp, lower, minn, maxx, One,
        PageIdx,
    )
    from concourse.dve_ops import has_src1
    from concourse.dve_table_gen import DveOpSpec

    def reg(name, spec, subdim=False):
        for existing in dvo.OPS:
            if existing.name == name:
                _OPS[name] = existing
                return
        row = dvo._CUSTOM_DVE_ROW_BASE + len(dvo.OPS)
        assert row < 0x20, "custom DVE row overflow"
        dvo._SUB_OPCODE_FOR_NAME[name] = row
        shas = {}
        for ver in ("v3", "v4"):
            s = DveOpSpec(name=name, opcode=row, uops=lower(spec, ver=ver),
                          rd1_en=has_src1(spec))
            shas[ver] = s.sha(ver)
        op = dvo.DveOp(name, spec, subdim, uops_sha=shas)
        dvo.OPS.append(op)
        dvo.CUSTOM_DVE_SPECS[name] = spec
        _OPS[name] = op

    # PPS: running sum of p*320 * 50 -> uint16
    def _pps_ref(in0, in1, s0, s1, imm2):
        r = np.cumsum(in0.astype(np.float32) * np.float32(imm2), axis=-1,
                      dtype=np.float32)
        return np.clip(np.rint(r), 0, 65535)

    reg("V7_PPS", Spec(body=scan(AluOp.ADD, Src0 * C2), reference=_pps_ref))

    # PVS: running sum of p*CS_V * (clip(pos16, c0, c1) + c2) -> uint16
    # pos16 = posr - 50.5; c0 = -50.5, c1 = -0.5, c2 = 51.5
    def _pvs_ref(in0, in1, s0, s1, imm2):
        t = np.clip(in1.astype(np.float32), s0, s1) + np.float32(imm2)
        r = np.cumsum(in0.astype(np.float32) * t, axis=-1, dtype=np.float32)
        return np.clip(np.rint(r), 0, 65535)

    reg("V7_PVS",
        Spec(body=scan(AluOp.ADD, Src0 * (minn(maxx(Src1, C0), C1) + C2)),
             reference=_pvs_ref))

    # VCO: (1 + 54*page) - clip(pos16, c0, c1) -> int16 (rint)
    def _vco_ref(in0, in1, s0, s1, imm2):
        S = in0.shape[1]
        pg = 1.0 + np.float32(imm2) * np.arange(S, dtype=np.float32)
        r = pg[None, :, None] - np.clip(in0.astype(np.float32), s0, s1)
        return np.clip(np.rint(r), -32768, 32767)

    reg("V7_VCO",
        Spec(body=PageIdx(One, C2) - minn(maxx(Src0, C0), C1),
             reference=_vco_ref),
        subdim=True)

    # JPF: runmax(dstp) * jconst
    def _jpf_ref(in0, in1, s0, s1, imm2):
        return np.maximum.accumulate(in0.astype(np.float32), axis=-1) \
            * in1.astype(np.float32)

    reg("V7_JPF", Spec(body=scan(AluOp.MAX, Src0) * Src1, reference=_jpf_ref))

    # RVF: jp - runmax(dstv) * c0
    def _rvf_ref(in0, in1, s0, s1, imm2):
        return in1.astype(np.float32) \
            - np.maximum.accumulate(in0.astype(np.float32), axis=-1) * s0

    reg("V7_RVF", Spec(body=Src1 - scan(AluOp.MAX, Src0) * C0,
                       reference=_rvf_ref))

    # FIN: (a - b) * c0   (stencil difference)
    def _fin_ref(in0, in1, s0, s1, imm2):
        return (in0.astype(np.float32) - in1.astype(np.float32)) * s0

    reg("V7_FIN", Spec(body=(Src0 - Src1) * C0, reference=_fin_ref))
    return _OPS


def _build(n_macros=M):
    ops = _register_ops()
    nc = bacc.Bacc()
    nrows = n_macros * MACRO_ROWS
    AluOp = mybir.AluOpType
    ACTF = mybir.ActivationFunctionType

    reward_in = nc.dram_tensor("reward", [nrows, 1], f32, kind="ExternalInput")
    probs_in = nc.dram_tensor("probs", [nrows, A], f32, kind="ExternalInput")
    nd_in = nc.dram_tensor("not_done", [nrows, 1], f32, kind="ExternalInput")
    jrow_in = nc.dram_tensor("jrow", [P, 4 * NE], f32, kind="ExternalInput")
    sct_in = nc.dram_tensor("sct_t", [16, nrows // G], f32,
                            kind="ExternalInput")
    aff_in = nc.dram_tensor("aff", [16, W], f32, kind="ExternalInput")
    vc0_in = nc.dram_tensor("vc0", [P, G], i16, kind="ExternalInput")
    out_t = nc.dram_tensor("out", [nrows, A], f32, kind="ExternalOutput")

    # row(m, p, g) = m*1024 + p*8 + g
    pr = probs_in[:].rearrange("(m p g) c -> m p (g c)", m=n_macros, p=P, g=G)
    outr = out_t[:].rearrange("(m p g) c -> m p (g c)", m=n_macros, p=P, g=G)


    with tile.TileContext(nc) as tc:
        with tc.tile_pool(name="consts", bufs=1) as cpool, \
             tc.tile_pool(name="work", bufs=3) as pool, \
             tc.tile_pool(name="dsts", bufs=3) as dpool, \
             tc.tile_pool(name="psum", bufs=3, space=MemorySpace.PSUM) as ppool:
            jrow = cpool.tile([P, 4 * NE], f32)
            nc.sync.dma_start(out=jrow[:], in_=jrow_in[:])
            aff = cpool.tile([16, W], f32)
            nc.sync.dma_start(out=aff[:], in_=aff_in[:])
            # all per-row scalars for all macros, transposed: [16, M*128]
            sct = cpool.tile([16, n_macros * P], f32)
            nc.sync.dma_start(out=sct[:], in_=sct_in[:])

            # warm up rotating buffers that carry persistent constants
            for b in range(3):
                pt = pool.tile([P, W], f32, name=f"pt32_w{b}", tag="pt32")
                nc.vector.memset(pt[:], 0)
            for b in range(3):
                vc = pool.tile([P, W], i16, name=f"vcols_w{b}", tag="vcols")
                vcr = vc[:].rearrange("p (g x) -> p g x", g=G)
                nc.sync.dma_start(
                    out=vcr[:, :, 0:1],
                    in_=vc0_in[:].rearrange("p (g o) -> p g o", o=1))

            jrow_r = jrow[:].rearrange("p (g x) -> p g x", g=4)
            prev = None
            for mi in range(n_macros + 1):
                if mi < n_macros:
                    ptile = pool.tile([P, G * A], f32, tag="ptile")
                    nc.sync.dma_start(out=ptile[:], in_=pr[mi])

                    pos_ps = ppool.tile([P, W], f32, tag="pos")
                    nc.tensor.matmul(pos_ps[:], sct[:, mi * P:(mi + 1) * P],
                                     aff[:], start=True, stop=True)

                    pt32 = pool.tile([P, W], f32, tag="pt32")
                    pt32_r = pt32[:].rearrange("p (g x) -> p g x", g=G)
                    nc.scalar.activation(pt32_r[:, :, 1:52], ptile[:],
                                         ACTF.Copy, bias=0.0, scale=CS_V)
                    pos32 = pool.tile([P, W], f32, tag="pos32")
                    nc.scalar.activation(pos32[:], pos_ps[:], ACTF.Copy,
                                         bias=-25.5, scale=1.0)

                    pps = pool.tile([P, W], u16, tag="pps")
                    for b in range(2):
                        h = slice(b * 208, (b + 1) * 208)
                        nc.vector._custom_dve(ops["V7_PPS"], out=pps[:, h],
                                              in0=pt32[:, h], imm2=RP)
                    pvs = pool.tile([P, W], u16, tag="pvs")
                    for q in range(4):
                        h = slice(q * 104, (q + 1) * 104)
                        nc.vector._custom_dve(ops["V7_PVS"], out=pvs[:, h],
                                              in0=pt32[:, h], in1=pos32[:, h],
                                              s0=-50.5, s1=-0.5, imm2=51.5)
                    vcols = pool.tile([P, W], i16, tag="vcols")
                    vcr = vcols[:].rearrange("p (g x) -> p g x", g=G)
                    p32r = pos32[:].rearrange("p (g x) -> p g x", g=G)
                    nc.vector._custom_dve(ops["V7_VCO"], out=vcr[:, :, 1:52],
                                          in0=p32r[:, :, 1:52],
                                          s0=-50.99, s1=-0.01, imm2=float(NE))

                    dstp = dpool.tile([P, WD], u16, tag="dstp")
                    nc.gpsimd.local_scatter(dstp[:], pps[:], vcols[:],
                                            channels=P, num_elems=WD,
                                            num_idxs=W)
                    dstv = dpool.tile([P, WD], u16, tag="dstv")
                    nc.gpsimd.local_scatter(dstv[:], pvs[:], vcols[:],
                                            channels=P, num_elems=WD,
                                            num_idxs=W)
                    cur = (dstp, dstv, mi)

                if mi > 0:
                    dstp0, dstv0, mj = prev
                    dstp_r = dstp0[:].rearrange("p (g x) -> p g x", g=G)
                    dstv_r = dstv0[:].rearrange("p (g x) -> p g x", g=G)
                    jpf = pool.tile([P, WD], f32, tag="jpf")
                    jpf_r = jpf[:].rearrange("p (g x) -> p g x", g=G)
                    for b in range(2):
                        gs = slice(b * 4, (b + 1) * 4)
                        nc.vector._custom_dve(ops["V7_JPF"],
                                              out=jpf_r[:, gs, :],
                                              in0=dstp_r[:, gs, ::-1],
                                              in1=jrow_r[:])
                    rtile = pool.tile([P, WD], f32, tag="rtile")
                    rt_r = rtile[:].rearrange("p (g x) -> p g x", g=G)
                    for q in range(4):
                        gs = slice(q * 2, (q + 1) * 2)
                        nc.vector._custom_dve(ops["V7_RVF"],
                                              out=rt_r[:, gs, :],
                                              in0=dstv_r[:, gs, ::-1],
                                              in1=jpf_r[:, gs, :],
                                              s0=1.0 / CS_V)

                    d1t = pool.tile([P, G * 53], f32, tag="d1t")
                    d1r = d1t[:].rearrange("p (g x) -> p g x", g=G)
                    nc.vector._custom_dve(ops["V7_FIN"], out=d1r[:],
                                          in0=rt_r[:, :, 1:54],
                                          in1=rt_r[:, :, 0:53], s0=1.0)
                    otile = pool.tile([P, G * A], f32, tag="otile")
                    otr = otile[:].rearrange("p (g x) -> p g x", g=G)
                    nc.vector._custom_dve(ops["V7_FIN"], out=otr[:],
                                          in0=d1r[:, :, 1:52],
                                          in1=d1r[:, :, 0:51], s0=1.0)
                    nc.scalar.dma_start(out=outr[mj], in_=otile[:])

                if mi < n_macros:
                    prev = cur
    nc.compile()
    return nc


_CONSTS = None


def _const_inputs():
    global _CONSTS
    if _CONSTS is None:
        jrow = np.tile((np.arange(NE, dtype=np.float32) / np.float32(CS_P)),
                       4)
        jrow = np.tile(jrow[None, :], (P, 1)).astype(np.float32)
        # aff[q, (g, c)]: q = 0..7 -> reward row of group q: coef 2.5
        #                 q = 8..15 -> nd row: coef 0.99*(c-1) - 24.75
        # (+25 global offset folded into the pos16 activation bias)
        aff = np.zeros((16, W), dtype=np.float32)
        cc = np.arange(52, dtype=np.float32)
        for g in range(G):
            aff[g, g * 52:(g + 1) * 52] = 2.5
            aff[8 + g, g * 52:(g + 1) * 52] = 0.99 * (cc - 1.0) - 24.75
        vc0 = np.tile((53 + NE * np.arange(G, dtype=np.int16))[None, :],
                      (P, 1)).astype(np.int16)
        _CONSTS = {"jrow": jrow, "aff": aff, "vc0": vc0}
    return _CONSTS


_PROGRAM = None


def make_in_maps(reward, probs, not_done):
    consts = _const_inputs()
    in_maps = []
    for c in range(N_CORES):
        sl = slice(c * ROWS, (c + 1) * ROWS)
        rt = reward[sl].reshape(M, P, G).transpose(2, 0, 1).reshape(G, -1)
        nt = not_done[sl].reshape(M, P, G).transpose(2, 0, 1).reshape(G, -1)
        sct_t = np.ascontiguousarray(np.concatenate([rt, nt], axis=0),
                                     dtype=np.float32)
        in_maps.append({
            "reward": np.ascontiguousarray(reward[sl]),
            "probs": np.ascontiguousarray(probs[sl]),
            "not_done": np.ascontiguousarray(not_done[sl]),
            "sct_t": sct_t,
            **consts,
        })
    return in_maps


def kernel(reward, probs, not_done):
    global _PROGRAM
    reward = np.ascontiguousarray(np.asarray(reward, dtype=np.float32))
    probs = np.ascontiguousarray(np.asarray(probs, dtype=np.float32))
    not_done = np.ascontiguousarray(np.asarray(not_done, dtype=np.float32))
    assert reward.shape == (BS, 1) and probs.shape == (BS, A)

    if _PROGRAM is None:
        _PROGRAM = _build(M)
    consts = _const_inputs()

    in_maps = make_in_maps(reward, probs, not_done)
    res = run_bass_kernel_spmd(_PROGRAM, in_maps, list(range(N_CORES)))
    out = np.empty((BS, A), dtype=np.float32)
    for c in range(N_CORES):
        out[c * ROWS:(c + 1) * ROWS] = res.results[c]["out"]
    return out


# revision 21
# speedup vs baseline: 1.1951x; 1.0361x over previous
"""Trainium2 Bass kernel for nn_CategoricalProjection (C51 categorical
projection / histogram binning), v7.

Math: out[j] = d2/dj2 R(j), R(y) = sum_a p_a relu(y - pos_a),
pos_a = clip(alpha + beta*a, 0, 50), alpha = 2.5 r + 25 - 24.75 nd,
beta = 0.99 nd.  R(y) = (y+1) P(c_y) - Vhat(c_y) with P = prefix(p),
Vhat = prefix(p*(pos+1)), c_y = #{a: pos_a < y} -- both prefixes taken as
GLOBAL running sums across group batches (constant offsets per group are
annihilated by the second difference; (y+1)P offsets are linear in y).

Per 128-row x 8-group macro:
  PE matmul (lhsT = [rT; ndT], const AFF) -> posext[p, (g,c)] (affine tile)
  Act: pt16 = 320*p (fp16), pos16 = posext - 50.5 (fp16)
  DVE custom scans: PPS (uint16, 4-group batches), PVS (int16, 2-group)
  DVE custom: vcols = (1 + 54g) - clip(pos16, -50.99, -0.01)  [rint -> int16]
  GPSIMD local_scatter (last-wins): dstp[vcols] = PPS, dstv[vcols] = PVS;
    col0 of vcols = const 53 + 54g (base slot, per-group re-seed)
  DVE fills (reversed windows): JP = runmax(dstp) * jconst(k = y+1),
    R = JP - runmax(dstv)/320
  DVE stencils: out = d2 R

Sharding: pure data-parallel over batch across 8 NeuronCores.
"""
import numpy as np

import concourse.bacc as bacc
import concourse.tile as tile
from concourse import mybir
from concourse.bass import MemorySpace
from concourse.bass_utils import run_bass_kernel_spmd

# ---- problem constants ----
BS = 524288
A = 51
N_CORES = 8
ROWS = BS // N_CORES          # 65536 rows per core
P = 128
G = 8                         # groups (rows) per partition per macro
MACRO_ROWS = P * G            # 1024
M = ROWS // MACRO_ROWS        # 64 macros

CS_V = 320.0                  # Vhat fixed-point scale (uint16, 4-group)
RP = 50.0                     # extra P scale: CS_P = CS_V * RP = 16000
CS_P = CS_V * RP              # uint16, 4-group batches
NE = 54                       # dst slots per group window
W = G * 52                    # 416: pps/pvs/vcols width
WD = G * NE                   # 432: dst width

f32 = mybir.dt.float32
f16 = mybir.dt.float16
i16 = mybir.dt.int16
u16 = mybir.dt.uint16

_OPS = {}


def _register_ops():
    if _OPS:
        return _OPS
    from concourse import dve_ops as dvo
    from concourse.dve_spec import (
        Spec, Src0, Src1, C0, C1, C2, scan, AluOp, lower, minn, maxx, One,
        Zero, Idx, PageIdx,
    )
    from concourse.dve_ops import has_src1
    from concourse.dve_table_gen import DveOpSpec

    def reg(name, spec, subdim=False):
        for existing in dvo.OPS:
            if existing.name == name:
                _OPS[name] = existing
                return
        row = dvo._CUSTOM_DVE_ROW_BASE + len(dvo.OPS)
        assert row < 0x20, "custom DVE row overflow"
        dvo._SUB_OPCODE_FOR_NAME[name] = row
        shas = {}
        for ver in ("v3", "v4"):
            s = DveOpSpec(name=name, opcode=row, uops=lower(spec, ver=ver),
                          rd1_en=has_src1(spec))
            shas[ver] = s.sha(ver)
        op = dvo.DveOp(name, spec, subdim, uops_sha=shas)
        dvo.OPS.append(op)
        dvo.CUSTOM_DVE_SPECS[name] = spec
        _OPS[name] = op

    # PPS: running sum of p*320 * 50 -> uint16
    def _pps_ref(in0, in1, s0, s1, imm2):
        r = np.cumsum(in0.astype(np.float32) * np.float32(imm2), axis=-1,
                      dtype=np.float32)
        return np.clip(np.rint(r), 0, 65535)

    reg("V7_PPS", Spec(body=scan(AluOp.ADD, Src0 * C2), reference=_pps_ref))

    # PVS: running sum of p*CS_V * (clip(pos16, c0, c1) + c2) -> uint16
    # pos16 = posr - 50.5; c0 = -50.5, c1 = -0.5, c2 = 51.5
    def _pvs_ref(in0, in1, s0, s1, imm2):
        t = np.clip(in1.astype(np.float32), s0, s1) + np.float32(imm2)
        r = np.cumsum(in0.astype(np.float32) * t, axis=-1, dtype=np.float32)
        return np.clip(np.rint(r), 0, 65535)

    reg("V7_PVS",
        Spec(body=scan(AluOp.ADD, Src0 * (minn(maxx(Src1, C0), C1) + C2)),
             reference=_pvs_ref))

    # VCO: (1 + 54*page) - clip(pos16, c0, c1) -> int16 (rint)
    def _vco_ref(in0, in1, s0, s1, imm2):
        S = in0.shape[1]
        pg = 1.0 + np.float32(imm2) * np.arange(S, dtype=np.float32)
        r = pg[None, :, None] - np.clip(in0.astype(np.float32), s0, s1)
        return np.clip(np.rint(r), -32768, 32767)

    reg("V7_VCO",
        Spec(body=PageIdx(One, C2) - minn(maxx(Src0, C0), C1),
             reference=_vco_ref),
        subdim=True)

    # JPF: runmax(dstp) * jconst
    def _jpf_ref(in0, in1, s0, s1, imm2):
        return np.maximum.accumulate(in0.astype(np.float32), axis=-1) \
            * in1.astype(np.float32)

    reg("V7_JPF", Spec(body=scan(AluOp.MAX, Src0) * Src1, reference=_jpf_ref))

    # RVF: jp - runmax(dstv) * c0
    def _rvf_ref(in0, in1, s0, s1, imm2):
        return in1.astype(np.float32) \
            - np.maximum.accumulate(in0.astype(np.float32), axis=-1) * s0

    reg("V7_RVF", Spec(body=Src1 - scan(AluOp.MAX, Src0) * C0,
                       reference=_rvf_ref))

    # FIN: (a - b) * c0   (stencil difference)
    def _fin_ref(in0, in1, s0, s1, imm2):
        return (in0.astype(np.float32) - in1.astype(np.float32)) * s0

    reg("V7_FIN", Spec(body=(Src0 - Src1) * C0, reference=_fin_ref))
    return _OPS


def _build(n_macros=M):
    ops = _register_ops()
    nc = bacc.Bacc()
    nrows = n_macros * MACRO_ROWS
    AluOp = mybir.AluOpType
    ACTF = mybir.ActivationFunctionType

    reward_in = nc.dram_tensor("reward", [nrows, 1], f32, kind="ExternalInput")
    probs_in = nc.dram_tensor("probs", [nrows, A], f32, kind="ExternalInput")
    nd_in = nc.dram_tensor("not_done", [nrows, 1], f32, kind="ExternalInput")
    jrow_in = nc.dram_tensor("jrow", [P, 4 * NE], f32, kind="ExternalInput")
    sct_in = nc.dram_tensor("sct_t", [16, nrows // G], f32,
                            kind="ExternalInput")
    aff_in = nc.dram_tensor("aff", [16, W], f32, kind="ExternalInput")
    vc0_in = nc.dram_tensor("vc0", [P, G], i16, kind="ExternalInput")
    out_t = nc.dram_tensor("out", [nrows, A], f32, kind="ExternalOutput")

    # row(m, p, g) = m*1024 + p*8 + g
    pr = probs_in[:].rearrange("(m p g) c -> m p (g c)", m=n_macros, p=P, g=G)
    outr = out_t[:].rearrange("(m p g) c -> m p (g c)", m=n_macros, p=P, g=G)


    with tile.TileContext(nc) as tc:
        with tc.tile_pool(name="consts", bufs=1) as cpool, \
             tc.tile_pool(name="work", bufs=3) as pool, \
             tc.tile_pool(name="dsts", bufs=3) as dpool, \
             tc.tile_pool(name="psum", bufs=3, space=MemorySpace.PSUM) as ppool:
            jrow = cpool.tile([P, 4 * NE], f32)
            nc.sync.dma_start(out=jrow[:], in_=jrow_in[:])
            aff = cpool.tile([16, W], f32)
            nc.sync.dma_start(out=aff[:], in_=aff_in[:])
            # all per-row scalars for all macros, transposed: [16, M*128]
            sct = cpool.tile([16, n_macros * P], f32)
            nc.sync.dma_start(out=sct[:], in_=sct_in[:])

            # warm up rotating buffers that carry persistent constants
            for b in range(3):
                pt = pool.tile([P, W], f32, name=f"pt32_w{b}", tag="pt32")
                nc.vector.memset(pt[:], 0)
            for b in range(3):
                vc = pool.tile([P, W], i16, name=f"vcols_w{b}", tag="vcols")
                vcr = vc[:].rearrange("p (g x) -> p g x", g=G)
                nc.sync.dma_start(
                    out=vcr[:, :, 0:1],
                    in_=vc0_in[:].rearrange("p (g o) -> p g o", o=1))

            jrow_r = jrow[:].rearrange("p (g x) -> p g x", g=4)
            prev = None
            for mi in range(n_macros + 1):
                if mi < n_macros:
                    ptile = pool.tile([P, G * A], f32, tag="ptile")
                    nc.sync.dma_start(out=ptile[:], in_=pr[mi])

                    pos_ps = ppool.tile([P, W], f32, tag="pos")
                    nc.tensor.matmul(pos_ps[:], sct[:, mi * P:(mi + 1) * P],
                                     aff[:], start=True, stop=True)

                    pt32 = pool.tile([P, W], f32, tag="pt32")
                    pt32_r = pt32[:].rearrange("p (g x) -> p g x", g=G)
                    nc.scalar.activation(pt32_r[:, :, 1:52], ptile[:],
                                         ACTF.Copy, bias=0.0, scale=CS_V)
                    pos32 = pool.tile([P, W], f32, tag="pos32")
                    nc.scalar.activation(pos32[:], pos_ps[:], ACTF.Copy,
                                         bias=-25.5, scale=1.0)

                    pps = pool.tile([P, W], u16, tag="pps")
                    for b in range(2):
                        h = slice(b * 208, (b + 1) * 208)
                        nc.vector._custom_dve(ops["V7_PPS"], out=pps[:, h],
                                              in0=pt32[:, h], imm2=RP)
                    pvs = pool.tile([P, W], u16, tag="pvs")
                    for b in range(2):
                        h = slice(b * 208, (b + 1) * 208)
                        nc.vector._custom_dve(ops["V7_PVS"], out=pvs[:, h],
                                              in0=pt32[:, h], in1=pos32[:, h],
                                              s0=-50.5, s1=-0.5, imm2=51.5)
                    vcols = pool.tile([P, W], i16, tag="vcols")
                    vcr = vcols[:].rearrange("p (g x) -> p g x", g=G)
                    p32r = pos32[:].rearrange("p (g x) -> p g x", g=G)
                    nc.vector._custom_dve(ops["V7_VCO"], out=vcr[:, :, 1:52],
                                          in0=p32r[:, :, 1:52],
                                          s0=-50.99, s1=-0.01, imm2=float(NE))

                    dstp = dpool.tile([P, WD], u16, tag="dstp")
                    nc.gpsimd.local_scatter(dstp[:], pps[:], vcols[:],
                                            channels=P, num_elems=WD,
                                            num_idxs=W)
                    dstv = dpool.tile([P, WD], u16, tag="dstv")
                    nc.gpsimd.local_scatter(dstv[:], pvs[:], vcols[:],
                                            channels=P, num_elems=WD,
                                            num_idxs=W)
                    cur = (dstp, dstv, mi)

                if mi > 0:
                    dstp0, dstv0, mj = prev
                    dstp_r = dstp0[:].rearrange("p (g x) -> p g x", g=G)
                    dstv_r = dstv0[:].rearrange("p (g x) -> p g x", g=G)
                    jpf = pool.tile([P, WD], f32, tag="jpf")
                    jpf_r = jpf[:].rearrange("p (g x) -> p g x", g=G)
                    for b in range(2):
                        gs = slice(b * 4, (b + 1) * 4)
                        nc.vector._custom_dve(ops["V7_JPF"],
                                              out=jpf_r[:, gs, :],
                                              in0=dstp_r[:, gs, ::-1],
                                              in1=jrow_r[:])
                    rtile = pool.tile([P, WD], f32, tag="rtile")
                    rt_r = rtile[:].rearrange("p (g x) -> p g x", g=G)
                    for b in range(2):
                        gs = slice(b * 4, (b + 1) * 4)
                        nc.vector._custom_dve(ops["V7_RVF"],
                                              out=rt_r[:, gs, :],
                                              in0=dstv_r[:, gs, ::-1],
                                              in1=jpf_r[:, gs, :],
                                              s0=1.0 / CS_V)

                    d1t = pool.tile([P, G * 53], f32, tag="d1t")
                    d1r = d1t[:].rearrange("p (g x) -> p g x", g=G)
                    nc.vector._custom_dve(ops["V7_FIN"], out=d1r[:],
                                          in0=rt_r[:, :, 1:54],
                                          in1=rt_r[:, :, 0:53], s0=1.0)
                    otile = pool.tile([P, G * A], f32, tag="otile")
                    otr = otile[:].rearrange("p (g x) -> p g x", g=G)
                    nc.vector._custom_dve(ops["V7_FIN"], out=otr[:],
                                          in0=d1r[:, :, 1:52],
                                          in1=d1r[:, :, 0:51], s0=1.0)
                    nc.scalar.dma_start(out=outr[mj], in_=otile[:])

                if mi < n_macros:
                    prev = cur
    nc.compile()
    return nc


_CONSTS = None


def _const_inputs():
    global _CONSTS
    if _CONSTS is None:
        jrow = np.tile((np.arange(NE, dtype=np.float32) / np.float32(CS_P)),
                       4)
        jrow = np.tile(jrow[None, :], (P, 1)).astype(np.float32)
        # aff[q, (g, c)]: q = 0..7 -> reward row of group q: coef 2.5
        #                 q = 8..15 -> nd row: coef 0.99*(c-1) - 24.75
        # (+25 global offset folded into the pos16 activation bias)
        aff = np.zeros((16, W), dtype=np.float32)
        cc = np.arange(52, dtype=np.float32)
        for g in range(G):
            aff[g, g * 52:(g + 1) * 52] = 2.5
            aff[8 + g, g * 52:(g + 1) * 52] = 0.99 * (cc - 1.0) - 24.75
        vc0 = np.tile((53 + NE * np.arange(G, dtype=np.int16))[None, :],
                      (P, 1)).astype(np.int16)
        _CONSTS = {"jrow": jrow, "aff": aff, "vc0": vc0}
    return _CONSTS


_PROGRAM = None


def make_in_maps(reward, probs, not_done):
    consts = _const_inputs()
    in_maps = []
    for c in range(N_CORES):
        sl = slice(c * ROWS, (c + 1) * ROWS)
        rt = reward[sl].reshape(M, P, G).transpose(2, 0, 1).reshape(G, -1)
        nt = not_done[sl].reshape(M, P, G).transpose(2, 0, 1).reshape(G, -1)
        sct_t = np.ascontiguousarray(np.concatenate([rt, nt], axis=0),
                                     dtype=np.float32)
        in_maps.append({
            "reward": np.ascontiguousarray(reward[sl]),
            "probs": np.ascontiguousarray(probs[sl]),
            "not_done": np.ascontiguousarray(not_done[sl]),
            "sct_t": sct_t,
            **consts,
        })
    return in_maps


def kernel(reward, probs, not_done):
    global _PROGRAM
    reward = np.ascontiguousarray(np.asarray(reward, dtype=np.float32))
    probs = np.ascontiguousarray(np.asarray(probs, dtype=np.float32))
    not_done = np.ascontiguousarray(np.asarray(not_done, dtype=np.float32))
    assert reward.shape == (BS, 1) and probs.shape == (BS, A)

    if _PROGRAM is None:
        _PROGRAM = _build(M)
    consts = _const_inputs()

    in_maps = make_in_maps(reward, probs, not_done)
    res = run_bass_kernel_spmd(_PROGRAM, in_maps, list(range(N_CORES)))
    out = np.empty((BS, A), dtype=np.float32)
    for c in range(N_CORES):
        out[c * ROWS:(c + 1) * ROWS] = res.results[c]["out"]
    return out
